# revision 10
# baseline (speedup 1.0000x reference)
"""Self-contained TP-over-heads DeepseekAttention kernel for 8 TRN2 cores.

Sharding: tensor-parallel across heads (4 heads/core) for Q/K/V, column
parallel for o_proj. Per core:
  - inputs are fused into 3 tensors (per-run dispatch overhead scales with
    buffer count/bytes): bf16 weight shards, a bf16 seq-shard of x^T plus
    rope tables, and a tiny f32 constants tensor.
  - hidden_states arrives seq-sharded (2.1MB/core); one on-device
    AllGather (~25us on one chip) rebuilds the full x^T, replacing 134MB
    of replicated host->device upload.
  - V then Q/K projections (bf16 matmuls, full PE rate), RoPE fused via a
    rotation matmul; Q^T/K^T/V stay resident in SBUF in bf16 (no spills).
  - attention per 512-query chunk: scores^T (bf16), exp on ACT, P@V
    accumulated in fp32 PSUM, softmax denominator via a vector tree sum +
    ones-matmul cross-partition reduction.
  - o_proj is column-parallel: the 4 local heads' A^T (bf16, 0.5MB) is
    AllGather-ed per chunk (overlapped with the next chunk's attention),
    then each core computes all rows x its 512 output columns with the
    head reduction in fp32 PSUM.
  - output is a bf16 [S, 512] column shard per core; host concat + cast.
"""

import numpy as np
import ml_dtypes

import concourse.bass as bass  # noqa: F401  (bass types used via bacc/tile)
import concourse.mybir as mybir
import concourse.tile as tile
from concourse import bacc

# problem shapes (hardcoded per contract)
S = 2048
H = 4096
NH = 32
D = 128
NC = 8
HPC = NH // NC          # 4 heads per core
DPC = HPC * D           # 512 head-dims per core
KT = H // 128           # 32 contraction tiles over hidden
SPC = S // NC           # 256 seq positions per upload shard
SCH = 512               # s-chunk for QK projections
NSC = S // SCH          # 4
ST = S // 128           # 16 s-tiles
QCH = 512               # q-chunk in attention
NQC = S // QCH          # 4
NKT = S // 128          # 16 k-tiles in attention
NPC = H // NC           # 512 output columns per core

f32 = mybir.dt.float32
f32r = mybir.dt.float32r
bf16 = mybir.dt.bfloat16
bf16_np = ml_dtypes.bfloat16

ROPE_THETA = 10000.0
SCALE = float(1.0 / np.sqrt(D))

_CACHE: dict = {}


def _build(sim_local_colls=False):
    nc = bacc.Bacc("TRN2", target_bir_lowering=False, debug=False, num_devices=NC)

    # ---- I/O (fused: per-run overhead scales with buffer count) ----
    # wts = [Wq^T | Wk^T | Wv^T | Wo^T] column-concatenated, this core's shards
    wts = nc.dram_tensor("wts", [KT, 128, 4 * DPC], bf16, kind="ExternalInput").ap()
    # xsc = x^T seq-shard as KT tiles, plus cos^T/sin^T shards as 2 extra tiles
    xsc = nc.dram_tensor("xsc", [KT + 2, 128, SPC], bf16, kind="ExternalInput").ap()
    # misc: rmat in [0:128,0:128]; row 128 and col 128 are all-ones
    misc = nc.dram_tensor("misc", [132, 132], f32r, kind="ExternalInput").ap()
    out_ext = nc.dram_tensor("out", [S, NPC], bf16, kind="ExternalOutput").ap()
    wq = wts[:, :, 0 * DPC:1 * DPC]
    wk = wts[:, :, 1 * DPC:2 * DPC]
    wv = wts[:, :, 2 * DPC:3 * DPC]
    wo = wts[:, :, 3 * DPC:4 * DPC]
    rmat = misc[0:128, 0:128]
    ones_col = misc[0:128, 128:129]
    ones_row = misc[128:129, 0:128]

    rg = [list(range(NC))]

    with tile.TileContext(nc) as tc:
        with (
            tc.tile_pool(name="dram", bufs=1, space="DRAM") as dram_pool,
            tc.tile_pool(name="persist", bufs=1) as persist,
        ):
            # internal DRAM: collective ins must not be kernel I/O
            xsc_i = dram_pool.tile([KT + 2, 128, SPC], bf16, name="xsc_i")
            xg = dram_pool.tile([NC, KT + 2, 128, SPC], bf16, name="xg",
                                addr_space="Local" if sim_local_colls else "Shared")
            at_d = [dram_pool.tile([HPC, 128, QCH], bf16, name=f"at_d{i}")
                    for i in range(NQC)]
            at_g = [dram_pool.tile([NC * HPC, 128, QCH], bf16, name=f"at_g{i}",
                                   addr_space="Local" if sim_local_colls else "Shared")
                    for i in range(NQC)]

            def allgather(in_t, out_t, out_slice):
                if sim_local_colls:
                    # timing proxy for TimelineSim only: same bytes via DMA
                    for r in range(NC):
                        nc.gpsimd.dma_start(out_slice(r), in_t[:])
                else:
                    nc.gpsimd.collective_compute(
                        "AllGather", mybir.AluOpType.bypass, replica_groups=rg,
                        ins=[in_t.opt()], outs=[out_t.opt()],
                    )

            nc.gpsimd.dma_start(xsc_i[:], xsc[:])
            allgather(xsc_i, xg, lambda r: xg[r])

            # persistent SBUF: Q^T/K^T (post-rope, bf16) and V (bf16)
            qk_sb = persist.tile([128, 2 * HPC, S], bf16, tag="qk")
            v_sb = persist.tile([128, ST, DPC], bf16, tag="v")

            with tc.tile_pool(name="wqk", bufs=1) as wqk_pool:
                wq_sb = wqk_pool.tile([128, KT, DPC], bf16, tag="wq")
                wk_sb = wqk_pool.tile([128, KT, DPC], bf16, tag="wk")

                # ====== Phase V: V projection =================================
                with (
                    tc.tile_pool(name="wv", bufs=1) as wv_pool,
                    tc.tile_pool(name="xt2", bufs=2) as xt2_pool,
                    tc.tile_pool(name="psB", bufs=2, space="PSUM") as psB,
                ):
                    wv_sb = wv_pool.tile([128, KT, DPC], bf16, tag="wv")
                    for g in range(4):  # split so first matmuls start early
                        nc.sync.dma_start(
                            wv_sb[:, 8 * g:8 * (g + 1), :],
                            wv.rearrange("k p n -> p k n")[:, 8 * g:8 * (g + 1), :],
                        )
                    # prefetch Wq/Wk on the scalar-engine HWDGE queue
                    for g in range(4):
                        nc.scalar.dma_start(
                            wq_sb[:, :, 128 * g:128 * (g + 1)],
                            wq.rearrange("k p n -> p k n")[:, :, 128 * g:128 * (g + 1)],
                        )
                        nc.scalar.dma_start(
                            wk_sb[:, :, 128 * g:128 * (g + 1)],
                            wk.rearrange("k p n -> p k n")[:, :, 128 * g:128 * (g + 1)],
                        )
                    for r in range(NC):  # one gathered shard = 2 s-tiles
                        x_sb = xt2_pool.tile([128, KT, SPC], bf16, tag="x2")
                        nc.sync.dma_start(
                            x_sb[:],
                            xg[r, 0:KT].rearrange("k p s -> p k s"),
                        )
                        for half in range(2):
                            st = 2 * r + half
                            ps = psB.tile([128, DPC], f32, tag="vp")
                            for kt in range(KT):
                                nc.tensor.matmul(
                                    ps[:],
                                    x_sb[:, kt, 128 * half:128 * (half + 1)],
                                    wv_sb[:, kt, :],
                                    start=(kt == 0), stop=(kt == KT - 1),
                                )
                            nc.scalar.copy(v_sb[:, st, :], ps[:])

                # ====== Phase QK: Q/K projections + RoPE (stay in SBUF) ======
                with (
                    tc.tile_pool(name="xt1", bufs=2) as xt1_pool,
                    tc.tile_pool(name="ropec", bufs=2) as rope_pool,
                    tc.tile_pool(name="rmp", bufs=1) as rm_pool,
                    tc.tile_pool(name="qktmp", bufs=2) as qktmp_pool,
                    tc.tile_pool(name="psA", bufs=2, space="PSUM") as psA,
                ):
                    rm_sb = rm_pool.tile([128, 128], f32r, tag="rm")
                    nc.sync.dma_start(rm_sb[:], rmat)
                    for sc in range(NSC):
                        s0 = sc * SCH
                        x_sb = xt1_pool.tile([128, KT, SCH], bf16, tag="x")
                        for half in range(2):
                            nc.sync.dma_start(
                                x_sb[:, :, 256 * half:256 * (half + 1)],
                                xg[2 * sc + half, 0:KT].rearrange(
                                    "k p s -> p k s"),
                            )
                        cs_bf = rope_pool.tile([128, 2, SCH], bf16, tag="csb")
                        for half in range(2):
                            nc.scalar.dma_start(
                                cs_bf[:, 0, 256 * half:256 * (half + 1)],
                                xg[2 * sc + half, KT, :, :],
                            )
                            nc.scalar.dma_start(
                                cs_bf[:, 1, 256 * half:256 * (half + 1)],
                                xg[2 * sc + half, KT + 1, :, :],
                            )
                        cos_sb = rope_pool.tile([128, SCH], f32, tag="cos")
                        sin_sb = rope_pool.tile([128, SCH], f32, tag="sin")
                        nc.scalar.copy(cos_sb[:], cs_bf[:, 0, :])
                        nc.scalar.copy(sin_sb[:], cs_bf[:, 1, :])
                        for pi, w_sb in ((0, wq_sb), (1, wk_sb)):
                            for h in range(HPC):
                                ps = psA.tile([128, SCH], f32, tag="proj")
                                for kt in range(KT):
                                    nc.tensor.matmul(
                                        ps[:],
                                        w_sb[:, kt, h * 128:(h + 1) * 128],
                                        x_sb[:, kt, :],
                                        start=(kt == 0),
                                        stop=(kt == KT - 1),
                                    )
                                raw = qktmp_pool.tile([128, SCH], f32r, tag="raw")
                                nc.scalar.copy(raw[:], ps[:])
                                psr = psA.tile([128, SCH], f32, tag="rot")
                                nc.tensor.matmul(psr[:], rm_sb[:], raw[:],
                                                 start=True, stop=True)
                                t1 = qktmp_pool.tile([128, SCH], f32, tag="t1")
                                nc.vector.tensor_mul(t1[:], raw[:], cos_sb[:])
                                t2 = qktmp_pool.tile([128, SCH], f32, tag="t2")
                                nc.vector.tensor_mul(t2[:], psr[:], sin_sb[:])
                                nc.vector.tensor_add(
                                    qk_sb[:, pi * HPC + h, s0:s0 + SCH],
                                    t1[:], t2[:],
                                )

            # ====== Phase attn + column-parallel o_proj ======================
            with (
                tc.tile_pool(name="wo", bufs=1) as wo_pool,
                tc.tile_pool(name="pt", bufs=8) as pt_pool,
                tc.tile_pool(name="tmp", bufs=1) as tmp_pool,
                tc.tile_pool(name="attnmisc", bufs=3) as misc_pool,
                tc.tile_pool(name="otp", bufs=2) as ot_pool,
                tc.tile_pool(name="atg", bufs=2) as atg_pool,
                tc.tile_pool(name="drain", bufs=4) as drain_pool,
                tc.tile_pool(name="psC", bufs=1, space="PSUM") as psC,
            ):
                wo_sb = wo_pool.tile([128, KT, NPC], bf16, tag="wo")
                for g in range(4):
                    nc.scalar.dma_start(
                        wo_sb[:, 8 * g:8 * (g + 1), :],
                        wo.rearrange("k p n -> p k n")[:, 8 * g:8 * (g + 1), :],
                    )
                oc_sb = misc_pool.tile([128, 1], f32r, tag="ones_c", bufs=1)
                or_sb = misc_pool.tile([1, 128], f32r, tag="ones_r", bufs=1)
                nc.sync.dma_start(oc_sb[:], ones_col)
                nc.sync.dma_start(or_sb[:], ones_row)

                def o_proj(qc):
                    # column-parallel o_proj for q-chunk qc (after its AG)
                    q0 = qc * QCH
                    at_sb = atg_pool.tile([128, NH, QCH], bf16, tag="atg")
                    for g in range(NH):
                        nc.sync.dma_start(at_sb[:, g, :], at_g[qc][g])
                    for qt_local in range(QCH // 128):
                        ps = psC.tile([128, NPC], f32, tag="op", bufs=2)
                        for g in range(NH):
                            nc.tensor.matmul(
                                ps[:],
                                at_sb[:, g,
                                      qt_local * 128:(qt_local + 1) * 128],
                                wo_sb[:, g, :],
                                start=(g == 0), stop=(g == NH - 1),
                            )
                        dr = drain_pool.tile([128, NPC], bf16, tag="dr")
                        nc.vector.tensor_copy(dr[:], ps[:])
                        nc.sync.dma_start(
                            out_ext[q0 + qt_local * 128:
                                    q0 + (qt_local + 1) * 128, :],
                            dr[:],
                        )

                def finalize(qc, h, ps_o, t_sum):
                    # denominator cross-partition sum -> broadcast -> recip;
                    # deferred one head so the PE queue never waits on the
                    # DVE tree sum (the two tiny matmuls would stall it)
                    ps_sum = psC.tile([1, QCH], f32, tag="sumbc", bufs=1)
                    nc.tensor.matmul(ps_sum[:], oc_sb[:], t_sum[:],
                                     start=True, stop=True)
                    sum_sb = misc_pool.tile([1, QCH], f32r, tag="sum_sb")
                    nc.vector.tensor_copy(sum_sb[:], ps_sum[:])
                    ps_bc = psC.tile([128, QCH], f32, tag="sumbc", bufs=1)
                    nc.tensor.matmul(ps_bc[:], or_sb[:], sum_sb[:],
                                     start=True, stop=True)
                    recip_sb = misc_pool.tile([128, QCH], f32, tag="recip")
                    nc.vector.reciprocal(recip_sb[:], ps_bc[:])
                    ot_t = ot_pool.tile([128, QCH], bf16, tag=f"ot{h}",
                                        name=f"ot{h}")
                    nc.vector.tensor_mul(ot_t[:], ps_o[:], recip_sb[:])
                    nc.scalar.dma_start(at_d[qc][h], ot_t[:])

                for qc in range(NQC):
                    q0 = qc * QCH
                    pending = None
                    for h in range(HPC):
                        # scores^T + exp, interleaved with attn@V accumulation
                        ps_o = psC.tile([128, QCH], f32, tag="vmm", bufs=2)
                        pts = []
                        for kt in range(NKT):
                            ps_s = psC.tile([128, QCH], f32, tag="scores",
                                            bufs=3)
                            nc.tensor.matmul(
                                ps_s[:],
                                qk_sb[:, HPC + h, kt * 128:(kt + 1) * 128],
                                qk_sb[:, h, q0:q0 + QCH],
                                start=True, stop=True,
                            )
                            pt = pt_pool.tile([128, QCH], bf16, tag="pt")
                            nc.scalar.activation(
                                pt[:], ps_s[:],
                                mybir.ActivationFunctionType.Exp, scale=SCALE,
                            )
                            pts.append(pt)
                            if kt >= 2:
                                kv = kt - 2
                                nc.tensor.matmul(
                                    ps_o[:],
                                    v_sb[:, kv, h * 128:(h + 1) * 128],
                                    pts[kv][:],
                                    start=(kv == 0), stop=False,
                                )
                        for kv in (NKT - 2, NKT - 1):
                            nc.tensor.matmul(
                                ps_o[:],
                                v_sb[:, kv, h * 128:(h + 1) * 128],
                                pts[kv][:],
                                start=False, stop=(kv == NKT - 1),
                            )

                        # denominator: batched tree sum of the 16 P^T tiles
                        tmp = tmp_pool.tile([128, 8, QCH], f32, tag="tr")
                        for i in range(8):
                            nc.vector.tensor_add(tmp[:, i, :],
                                                 pts[2 * i][:], pts[2 * i + 1][:])
                        nc.vector.tensor_add(tmp[:, 0:4, :],
                                             tmp[:, 0:4, :], tmp[:, 4:8, :])
                        nc.vector.tensor_add(tmp[:, 0:2, :],
                                             tmp[:, 0:2, :], tmp[:, 2:4, :])
                        t_sum = misc_pool.tile([128, QCH], f32r, tag="tsum",
                                               bufs=2)
                        nc.vector.tensor_add(t_sum[:], tmp[:, 0, :], tmp[:, 1, :])

                        if pending is not None:
                            finalize(qc, *pending)
                        pending = (h, ps_o, t_sum)
                    finalize(qc, *pending)

                    # gather all 32 heads' A^T for this q-chunk
                    allgather(at_d[qc], at_g[qc],
                              lambda r, qc=qc: at_g[qc][HPC * r:HPC * (r + 1)])
                    # o_proj for the PREVIOUS chunk: its AG has had a full
                    # attention chunk's compute to complete, so the PE does
                    # not stall on the collective.
                    if qc > 0:
                        o_proj(qc - 1)
                o_proj(NQC - 1)

    nc.compile()
    return nc


def _host_prep(positions, hidden_states, Wq, Wk, Wv, Wo):
    X = np.asarray(hidden_states, dtype=np.float32).reshape(S, H)
    # [KT+2, 128, S]: x^T tiles then cos^T / sin^T rows (bf16)
    xsc_full = np.empty((KT + 2, 128, S), bf16_np)
    xsc_full[:KT] = X.T.astype(bf16_np).reshape(KT, 128, S)

    pos = np.asarray(positions).astype(np.float32)
    inv_freq = (1.0 / (ROPE_THETA ** (np.arange(0, D, 2, dtype=np.float32) / D)))
    freqs = pos[:, None] * inv_freq[None, :]
    emb = np.concatenate([freqs, freqs], axis=-1)        # [S, D]
    xsc_full[KT] = np.cos(emb).T.astype(bf16_np)
    xsc_full[KT + 1] = np.sin(emb).T.astype(bf16_np)

    mi = np.zeros((132, 132), np.float32)
    idx = np.arange(64)
    mi[64 + idx, idx] = -1.0   # rmat: out[0:64]  = -in[64:128]
    mi[idx, 64 + idx] = 1.0    # rmat: out[64:128] = in[0:64]
    mi[0:128, 128] = 1.0       # ones_col
    mi[128, 0:128] = 1.0       # ones_row

    # fused weight shards: [Wq^T | Wk^T | Wv^T | Wo^T] per core
    Wq = np.asarray(Wq, dtype=np.float32)
    Wk = np.asarray(Wk, dtype=np.float32)
    Wv = np.asarray(Wv, dtype=np.float32)
    Wo = np.asarray(Wo, dtype=np.float32)

    in_maps = []
    for c in range(NC):
        sl = slice(DPC * c, DPC * (c + 1))
        ssl = slice(SPC * c, SPC * (c + 1))
        wts_c = np.empty((H, 4 * DPC), bf16_np)
        wts_c[:, 0 * DPC:1 * DPC] = Wq[sl, :].T.astype(bf16_np)
        wts_c[:, 1 * DPC:2 * DPC] = Wk[sl, :].T.astype(bf16_np)
        wts_c[:, 2 * DPC:3 * DPC] = Wv[sl, :].T.astype(bf16_np)
        wts_c[:, 3 * DPC:4 * DPC] = Wo[sl, :].T.astype(bf16_np)
        in_maps.append({
            "wts": wts_c.reshape(KT, 128, 4 * DPC),
            "xsc": np.ascontiguousarray(xsc_full[:, :, ssl]),
            "misc": mi,
        })
    return in_maps


def _make_runner(nc):
    """Build a reusable jitted SPMD runner for the compiled module.

    Mirrors concourse.bass2jax.run_bass_via_pjrt but caches the jitted
    function so repeat calls skip re-tracing, and exposes staging helpers
    so benchmarks can pre-place inputs on device.
    """
    import jax
    from jax.sharding import Mesh, PartitionSpec, NamedSharding
    from jax.experimental.shard_map import shard_map
    from concourse.bass2jax import (
        install_neuronx_cc_hook, _bass_exec_p, partition_id_tensor,
    )

    install_neuronx_cc_hook()
    partition_name = nc.partition_id_tensor.name if nc.partition_id_tensor else None

    in_names, out_names, out_avals, out_shapes = [], [], [], []
    for alloc in nc.m.functions[0].allocations:
        if not isinstance(alloc, mybir.MemoryLocationSet):
            continue
        name = alloc.memorylocations[0].name
        if alloc.kind == "ExternalInput":
            if name != partition_name:
                in_names.append(name)
        elif alloc.kind == "ExternalOutput":
            shape = tuple(alloc.tensor_shape)
            dtype = mybir.dt.np(alloc.dtype)
            out_names.append(name)
            out_avals.append(jax.core.ShapedArray(shape, dtype))
            out_shapes.append((shape, dtype))

    n_params = len(in_names)
    n_outs = len(out_avals)
    all_names = list(in_names) + list(out_names)
    if partition_name is not None:
        all_names.append(partition_name)
    donate = tuple(range(n_params, n_params + n_outs))

    def _body(*args):
        operands = list(args)
        if partition_name is not None:
            operands.append(partition_id_tensor())
        outs = _bass_exec_p.bind(
            *operands,
            out_avals=tuple(out_avals),
            in_names=tuple(all_names),
            out_names=tuple(out_names),
            lowering_input_output_aliases=(),
            sim_require_finite=True,
            sim_require_nnan=True,
            nc=nc,
        )
        return tuple(outs)

    devices = jax.devices()[:NC]
    mesh = Mesh(np.asarray(devices), ("core",))
    in_specs = (PartitionSpec("core"),) * (n_params + n_outs)
    out_specs = (PartitionSpec("core"),) * len(out_names)
    sharded = jax.jit(
        shard_map(_body, mesh=mesh, in_specs=in_specs,
                  out_specs=out_specs, check_rep=False),
        donate_argnums=donate,
        keep_unused=True,
    )
    sharding = NamedSharding(mesh, PartitionSpec("core"))

    import jax.numpy as jnp

    zero_fns = [
        jax.jit(
            (lambda shape=shape, dtype=dtype:
             jnp.zeros((NC * shape[0], *shape[1:]), dtype)),
            out_shardings=sharding,
        )
        for shape, dtype in out_shapes
    ]

    class Runner:
        in_names = None
        out_names = None

        def stage(self, in_maps):
            import jax as _jax
            concat = [
                np.concatenate([np.asarray(in_maps[c][name])
                                for c in range(NC)], axis=0)
                for name in in_names
            ]
            staged = [_jax.device_put(a, sharding) for a in concat]
            _jax.block_until_ready(staged)
            return staged

        def make_zeros(self):
            return [zf() for zf in zero_fns]

        def run_async(self, staged, zs):
            return sharded(*staged, *zs)

        def run(self, staged):
            import jax as _jax
            outs = sharded(*staged, *self.make_zeros())
            _jax.block_until_ready(outs)
            return outs

    r = Runner()
    r.in_names = in_names
    r.out_names = out_names
    r.sharding = sharding
    return r


def _assemble(out_arr):
    """[NC*S, NPC] device/np array -> full [1, S, H] f32 output."""
    a = np.asarray(out_arr).reshape(NC, S, NPC)
    return a.transpose(1, 0, 2).reshape(1, S, H).astype(np.float32)


def kernel(positions, hidden_states, Wq, Wk, Wv, Wo):
    if "nc" not in _CACHE:
        _CACHE["nc"] = _build()
        _CACHE["runner"] = _make_runner(_CACHE["nc"])
    runner = _CACHE["runner"]
    in_maps = _host_prep(positions, hidden_states, Wq, Wk, Wv, Wo)
    staged = runner.stage(in_maps)
    outs = runner.run(staged)
    return _assemble(outs[0])


# revision 11
# speedup vs baseline: 1.0003x; 1.0003x over previous
"""Self-contained TP-over-heads DeepseekAttention kernel for 8 TRN2 cores.

Sharding: tensor-parallel across heads (4 heads/core) for Q/K/V, column
parallel for o_proj. Per core:
  - inputs are fused into 3 tensors (per-run dispatch overhead scales with
    buffer count/bytes): bf16 weight shards, a bf16 seq-shard of x^T plus
    rope tables, and a tiny f32 constants tensor.
  - hidden_states arrives seq-sharded (2.1MB/core); one on-device
    AllGather (~25us on one chip) rebuilds the full x^T, replacing 134MB
    of replicated host->device upload.
  - V then Q/K projections (bf16 matmuls, full PE rate), RoPE fused via a
    rotation matmul; Q^T/K^T/V stay resident in SBUF in bf16 (no spills).
  - attention per 512-query chunk: scores^T (bf16), exp on ACT, P@V
    accumulated in fp32 PSUM, softmax denominator via a vector tree sum +
    ones-matmul cross-partition reduction.
  - o_proj is column-parallel: the 4 local heads' A^T (bf16, 0.5MB) is
    AllGather-ed per chunk (overlapped with the next chunk's attention),
    then each core computes all rows x its 512 output columns with the
    head reduction in fp32 PSUM.
  - output is a bf16 [S, 512] column shard per core; host concat + cast.
"""

import numpy as np
import ml_dtypes

import concourse.bass as bass  # noqa: F401  (bass types used via bacc/tile)
import concourse.mybir as mybir
import concourse.tile as tile
from concourse import bacc

# problem shapes (hardcoded per contract)
S = 2048
H = 4096
NH = 32
D = 128
NC = 8
HPC = NH // NC          # 4 heads per core
DPC = HPC * D           # 512 head-dims per core
KT = H // 128           # 32 contraction tiles over hidden
SPC = S // NC           # 256 seq positions per upload shard
SCH = 512               # s-chunk for QK projections
NSC = S // SCH          # 4
ST = S // 128           # 16 s-tiles
QCH = 512               # q-chunk in attention
NQC = S // QCH          # 4
NKT = S // 128          # 16 k-tiles in attention
NPC = H // NC           # 512 output columns per core

f32 = mybir.dt.float32
f32r = mybir.dt.float32r
bf16 = mybir.dt.bfloat16
bf16_np = ml_dtypes.bfloat16

ROPE_THETA = 10000.0
SCALE = float(1.0 / np.sqrt(D))

_CACHE: dict = {}


def _build(sim_local_colls=False):
    nc = bacc.Bacc("TRN2", target_bir_lowering=False, debug=False, num_devices=NC)

    # ---- I/O (fused: per-run overhead scales with buffer count) ----
    # wts = [Wq^T | Wk^T | Wv^T | Wo^T] column-concatenated, this core's shards
    wts = nc.dram_tensor("wts", [KT, 128, 4 * DPC], bf16, kind="ExternalInput").ap()
    # xsc = x^T seq-shard as KT tiles, plus cos^T/sin^T shards as 2 extra tiles
    xsc = nc.dram_tensor("xsc", [KT + 2, 128, SPC], bf16, kind="ExternalInput").ap()
    # misc: rmat in [0:128,0:128]; row 128 and col 128 are all-ones
    misc = nc.dram_tensor("misc", [132, 132], f32r, kind="ExternalInput").ap()
    out_ext = nc.dram_tensor("out", [S, NPC], bf16, kind="ExternalOutput").ap()
    wq = wts[:, :, 0 * DPC:1 * DPC]
    wk = wts[:, :, 1 * DPC:2 * DPC]
    wv = wts[:, :, 2 * DPC:3 * DPC]
    wo = wts[:, :, 3 * DPC:4 * DPC]
    rmat = misc[0:128, 0:128]
    ones_col = misc[0:128, 128:129]
    ones_row = misc[128:129, 0:128]

    rg = [list(range(NC))]

    with tile.TileContext(nc) as tc:
        with (
            tc.tile_pool(name="dram", bufs=1, space="DRAM") as dram_pool,
            tc.tile_pool(name="persist", bufs=1) as persist,
        ):
            # internal DRAM: collective ins must not be kernel I/O
            xsc_i = dram_pool.tile([KT + 2, 128, SPC], bf16, name="xsc_i")
            xg = dram_pool.tile([NC, KT + 2, 128, SPC], bf16, name="xg",
                                addr_space="Local" if sim_local_colls else "Shared")
            at_d = [dram_pool.tile([HPC, 128, QCH], bf16, name=f"at_d{i}")
                    for i in range(NQC)]
            at_g = [dram_pool.tile([NC * HPC, 128, QCH], bf16, name=f"at_g{i}",
                                   addr_space="Local" if sim_local_colls else "Shared")
                    for i in range(NQC)]

            def allgather(in_t, out_t, out_slice):
                if sim_local_colls:
                    # timing proxy for TimelineSim only: same bytes via DMA
                    for r in range(NC):
                        nc.gpsimd.dma_start(out_slice(r), in_t[:])
                else:
                    nc.gpsimd.collective_compute(
                        "AllGather", mybir.AluOpType.bypass, replica_groups=rg,
                        ins=[in_t.opt()], outs=[out_t.opt()],
                    )

            nc.gpsimd.dma_start(xsc_i[:], xsc[:])
            allgather(xsc_i, xg, lambda r: xg[r])

            # persistent SBUF: Q^T/K^T (post-rope, bf16) and V (bf16)
            qk_sb = persist.tile([128, 2 * HPC, S], bf16, tag="qk")
            v_sb = persist.tile([128, ST, DPC], bf16, tag="v")

            with tc.tile_pool(name="wqk", bufs=1) as wqk_pool:
                wq_sb = wqk_pool.tile([128, KT, DPC], bf16, tag="wq")
                wk_sb = wqk_pool.tile([128, KT, DPC], bf16, tag="wk")

                # ====== Phase V: V projection =================================
                with (
                    tc.tile_pool(name="wv", bufs=1) as wv_pool,
                    tc.tile_pool(name="xt2", bufs=2) as xt2_pool,
                    tc.tile_pool(name="psB", bufs=2, space="PSUM") as psB,
                ):
                    wv_sb = wv_pool.tile([128, KT, DPC], bf16, tag="wv")
                    for g in range(4):  # split so first matmuls start early
                        nc.sync.dma_start(
                            wv_sb[:, 8 * g:8 * (g + 1), :],
                            wv.rearrange("k p n -> p k n")[:, 8 * g:8 * (g + 1), :],
                        )
                    # prefetch Wq/Wk on the scalar-engine HWDGE queue
                    for g in range(4):
                        nc.scalar.dma_start(
                            wq_sb[:, :, 128 * g:128 * (g + 1)],
                            wq.rearrange("k p n -> p k n")[:, :, 128 * g:128 * (g + 1)],
                        )
                        nc.scalar.dma_start(
                            wk_sb[:, :, 128 * g:128 * (g + 1)],
                            wk.rearrange("k p n -> p k n")[:, :, 128 * g:128 * (g + 1)],
                        )
                    for r in range(NC):  # one gathered shard = 2 s-tiles
                        x_sb = xt2_pool.tile([128, KT, SPC], bf16, tag="x2")
                        nc.sync.dma_start(
                            x_sb[:],
                            xg[r, 0:KT].rearrange("k p s -> p k s"),
                        )
                        for half in range(2):
                            st = 2 * r + half
                            ps = psB.tile([128, DPC], f32, tag="vp")
                            for kt in range(KT):
                                nc.tensor.matmul(
                                    ps[:],
                                    x_sb[:, kt, 128 * half:128 * (half + 1)],
                                    wv_sb[:, kt, :],
                                    start=(kt == 0), stop=(kt == KT - 1),
                                )
                            nc.scalar.copy(v_sb[:, st, :], ps[:])

                # ====== Phase QK: Q/K projections + RoPE (stay in SBUF) ======
                with (
                    tc.tile_pool(name="xt1", bufs=2) as xt1_pool,
                    tc.tile_pool(name="ropec", bufs=2) as rope_pool,
                    tc.tile_pool(name="rmp", bufs=1) as rm_pool,
                    tc.tile_pool(name="qktmp", bufs=2) as qktmp_pool,
                    tc.tile_pool(name="psA", bufs=2, space="PSUM") as psA,
                ):
                    rm_sb = rm_pool.tile([128, 128], f32r, tag="rm")
                    nc.sync.dma_start(rm_sb[:], rmat)
                    for sc in range(NSC):
                        s0 = sc * SCH
                        x_sb = xt1_pool.tile([128, KT, SCH], bf16, tag="x")
                        for half in range(2):
                            nc.sync.dma_start(
                                x_sb[:, :, 256 * half:256 * (half + 1)],
                                xg[2 * sc + half, 0:KT].rearrange("k p s -> p k s"),
                            )
                        cs_bf = rope_pool.tile([128, 2, SCH], bf16, tag="csb")
                        for half in range(2):
                            nc.scalar.dma_start(
                                cs_bf[:, 0, 256 * half:256 * (half + 1)],
                                xg[2 * sc + half, KT, :, :],
                            )
                            nc.scalar.dma_start(
                                cs_bf[:, 1, 256 * half:256 * (half + 1)],
                                xg[2 * sc + half, KT + 1, :, :],
                            )
                        cos_sb = rope_pool.tile([128, SCH], f32, tag="cos")
                        sin_sb = rope_pool.tile([128, SCH], f32, tag="sin")
                        nc.scalar.copy(cos_sb[:], cs_bf[:, 0, :])
                        nc.scalar.copy(sin_sb[:], cs_bf[:, 1, :])
                        for pi, w_sb in ((0, wq_sb), (1, wk_sb)):
                            for h in range(HPC):
                                ps = psA.tile([128, SCH], f32, tag="proj")
                                for kt in range(KT):
                                    nc.tensor.matmul(
                                        ps[:],
                                        w_sb[:, kt, h * 128:(h + 1) * 128],
                                        x_sb[:, kt, :],
                                        start=(kt == 0),
                                        stop=(kt == KT - 1),
                                    )
                                raw = qktmp_pool.tile([128, SCH], f32r, tag="raw")
                                nc.scalar.copy(raw[:], ps[:])
                                psr = psA.tile([128, SCH], f32, tag="rot")
                                nc.tensor.matmul(psr[:], rm_sb[:], raw[:],
                                                 start=True, stop=True)
                                t1 = qktmp_pool.tile([128, SCH], f32, tag="t1")
                                nc.vector.tensor_mul(t1[:], raw[:], cos_sb[:])
                                t2 = qktmp_pool.tile([128, SCH], f32, tag="t2")
                                nc.vector.tensor_mul(t2[:], psr[:], sin_sb[:])
                                nc.vector.tensor_add(
                                    qk_sb[:, pi * HPC + h, s0:s0 + SCH],
                                    t1[:], t2[:],
                                )

            # ====== Phase attn + column-parallel o_proj ======================
            with (
                tc.tile_pool(name="wo", bufs=1) as wo_pool,
                tc.tile_pool(name="pt", bufs=8) as pt_pool,
                tc.tile_pool(name="tmp", bufs=1) as tmp_pool,
                tc.tile_pool(name="attnmisc", bufs=3) as misc_pool,
                tc.tile_pool(name="otp", bufs=2) as ot_pool,
                tc.tile_pool(name="atg", bufs=2) as atg_pool,
                tc.tile_pool(name="drain", bufs=4) as drain_pool,
                tc.tile_pool(name="psC", bufs=1, space="PSUM") as psC,
            ):
                wo_sb = wo_pool.tile([128, KT, NPC], bf16, tag="wo")
                for g in range(4):
                    nc.scalar.dma_start(
                        wo_sb[:, 8 * g:8 * (g + 1), :],
                        wo.rearrange("k p n -> p k n")[:, 8 * g:8 * (g + 1), :],
                    )
                oc_sb = misc_pool.tile([128, 1], f32r, tag="ones_c", bufs=1)
                or_sb = misc_pool.tile([1, 128], f32r, tag="ones_r", bufs=1)
                nc.sync.dma_start(oc_sb[:], ones_col)
                nc.sync.dma_start(or_sb[:], ones_row)

                def o_proj(qc):
                    # column-parallel o_proj for q-chunk qc (after its AG)
                    q0 = qc * QCH
                    at_sb = atg_pool.tile([128, NH, QCH], bf16, tag="atg")
                    for g in range(NH):
                        nc.sync.dma_start(at_sb[:, g, :], at_g[qc][g])
                    for qt_local in range(QCH // 128):
                        ps = psC.tile([128, NPC], f32, tag="op", bufs=2)
                        for g in range(NH):
                            nc.tensor.matmul(
                                ps[:],
                                at_sb[:, g,
                                      qt_local * 128:(qt_local + 1) * 128],
                                wo_sb[:, g, :],
                                start=(g == 0), stop=(g == NH - 1),
                            )
                        dr = drain_pool.tile([128, NPC], bf16, tag="dr")
                        nc.vector.tensor_copy(dr[:], ps[:])
                        nc.sync.dma_start(
                            out_ext[q0 + qt_local * 128:
                                    q0 + (qt_local + 1) * 128, :],
                            dr[:],
                        )

                for qc in range(NQC):
                    q0 = qc * QCH
                    for h in range(HPC):
                        # scores^T + exp, interleaved with attn@V accumulation
                        ps_o = psC.tile([128, QCH], f32, tag="vmm", bufs=2)
                        pts = []
                        for kt in range(NKT):
                            ps_s = psC.tile([128, QCH], f32, tag="scores",
                                            bufs=3)
                            nc.tensor.matmul(
                                ps_s[:],
                                qk_sb[:, HPC + h, kt * 128:(kt + 1) * 128],
                                qk_sb[:, h, q0:q0 + QCH],
                                start=True, stop=True,
                            )
                            pt = pt_pool.tile([128, QCH], bf16, tag="pt")
                            nc.scalar.activation(
                                pt[:], ps_s[:],
                                mybir.ActivationFunctionType.Exp, scale=SCALE,
                            )
                            pts.append(pt)
                            if kt >= 2:
                                kv = kt - 2
                                nc.tensor.matmul(
                                    ps_o[:],
                                    v_sb[:, kv, h * 128:(h + 1) * 128],
                                    pts[kv][:],
                                    start=(kv == 0), stop=False,
                                )
                        for kv in (NKT - 2, NKT - 1):
                            nc.tensor.matmul(
                                ps_o[:],
                                v_sb[:, kv, h * 128:(h + 1) * 128],
                                pts[kv][:],
                                start=False, stop=(kv == NKT - 1),
                            )

                        # denominator: batched tree sum of the 16 P^T tiles
                        tmp = tmp_pool.tile([128, 8, QCH], f32, tag="tr")
                        for i in range(8):
                            nc.vector.tensor_add(tmp[:, i, :],
                                                 pts[2 * i][:], pts[2 * i + 1][:])
                        nc.vector.tensor_add(tmp[:, 0:4, :],
                                             tmp[:, 0:4, :], tmp[:, 4:8, :])
                        nc.vector.tensor_add(tmp[:, 0:2, :],
                                             tmp[:, 0:2, :], tmp[:, 2:4, :])
                        t_sum = misc_pool.tile([128, QCH], f32r, tag="tsum",
                                               bufs=2)
                        nc.vector.tensor_add(t_sum[:], tmp[:, 0, :], tmp[:, 1, :])

                        # cross-partition sum -> broadcast -> reciprocal
                        ps_sum = psC.tile([1, QCH], f32, tag="sumbc", bufs=1)
                        nc.tensor.matmul(ps_sum[:], oc_sb[:], t_sum[:],
                                         start=True, stop=True)
                        sum_sb = misc_pool.tile([1, QCH], f32r, tag="sum_sb")
                        nc.vector.tensor_copy(sum_sb[:], ps_sum[:])
                        ps_bc = psC.tile([128, QCH], f32, tag="sumbc", bufs=1)
                        nc.tensor.matmul(ps_bc[:], or_sb[:], sum_sb[:],
                                         start=True, stop=True)
                        recip_sb = misc_pool.tile([128, QCH], f32, tag="recip")
                        nc.vector.reciprocal(recip_sb[:], ps_bc[:])

                        ot_t = ot_pool.tile([128, QCH], bf16, tag=f"ot{h}",
                                            name=f"ot{h}")
                        nc.vector.tensor_mul(ot_t[:], ps_o[:], recip_sb[:])
                        nc.scalar.dma_start(at_d[qc][h], ot_t[:])

                    # gather all 32 heads' A^T for this q-chunk
                    allgather(at_d[qc], at_g[qc],
                              lambda r, qc=qc: at_g[qc][HPC * r:HPC * (r + 1)])
                    # o_proj for the PREVIOUS chunk: its AG has had a full
                    # attention chunk's compute to complete, so the PE does
                    # not stall on the collective.
                    if qc > 0:
                        o_proj(qc - 1)
                o_proj(NQC - 1)

    nc.compile()
    return nc


def _host_prep(positions, hidden_states, Wq, Wk, Wv, Wo):
    X = np.asarray(hidden_states, dtype=np.float32).reshape(S, H)
    # [KT+2, 128, S]: x^T tiles then cos^T / sin^T rows (bf16)
    xsc_full = np.empty((KT + 2, 128, S), bf16_np)
    xsc_full[:KT] = X.T.astype(bf16_np).reshape(KT, 128, S)

    pos = np.asarray(positions).astype(np.float32)
    inv_freq = (1.0 / (ROPE_THETA ** (np.arange(0, D, 2, dtype=np.float32) / D)))
    freqs = pos[:, None] * inv_freq[None, :]
    emb = np.concatenate([freqs, freqs], axis=-1)        # [S, D]
    xsc_full[KT] = np.cos(emb).T.astype(bf16_np)
    xsc_full[KT + 1] = np.sin(emb).T.astype(bf16_np)

    mi = np.zeros((132, 132), np.float32)
    idx = np.arange(64)
    mi[64 + idx, idx] = -1.0   # rmat: out[0:64]  = -in[64:128]
    mi[idx, 64 + idx] = 1.0    # rmat: out[64:128] = in[0:64]
    mi[0:128, 128] = 1.0       # ones_col
    mi[128, 0:128] = 1.0       # ones_row

    # fused weight shards: [Wq^T | Wk^T | Wv^T | Wo^T] per core
    Wq = np.asarray(Wq, dtype=np.float32)
    Wk = np.asarray(Wk, dtype=np.float32)
    Wv = np.asarray(Wv, dtype=np.float32)
    Wo = np.asarray(Wo, dtype=np.float32)

    in_maps = []
    for c in range(NC):
        sl = slice(DPC * c, DPC * (c + 1))
        ssl = slice(SPC * c, SPC * (c + 1))
        wts_c = np.empty((H, 4 * DPC), bf16_np)
        wts_c[:, 0 * DPC:1 * DPC] = Wq[sl, :].T.astype(bf16_np)
        wts_c[:, 1 * DPC:2 * DPC] = Wk[sl, :].T.astype(bf16_np)
        wts_c[:, 2 * DPC:3 * DPC] = Wv[sl, :].T.astype(bf16_np)
        wts_c[:, 3 * DPC:4 * DPC] = Wo[sl, :].T.astype(bf16_np)
        in_maps.append({
            "wts": wts_c.reshape(KT, 128, 4 * DPC),
            "xsc": np.ascontiguousarray(xsc_full[:, :, ssl]),
            "misc": mi,
        })
    return in_maps


def _make_runner(nc):
    """Build a reusable jitted SPMD runner for the compiled module.

    Mirrors concourse.bass2jax.run_bass_via_pjrt but caches the jitted
    function so repeat calls skip re-tracing, and exposes staging helpers
    so benchmarks can pre-place inputs on device.
    """
    import jax
    from jax.sharding import Mesh, PartitionSpec, NamedSharding
    from jax.experimental.shard_map import shard_map
    from concourse.bass2jax import (
        install_neuronx_cc_hook, _bass_exec_p, partition_id_tensor,
    )

    install_neuronx_cc_hook()
    partition_name = nc.partition_id_tensor.name if nc.partition_id_tensor else None

    in_names, out_names, out_avals, out_shapes = [], [], [], []
    for alloc in nc.m.functions[0].allocations:
        if not isinstance(alloc, mybir.MemoryLocationSet):
            continue
        name = alloc.memorylocations[0].name
        if alloc.kind == "ExternalInput":
            if name != partition_name:
                in_names.append(name)
        elif alloc.kind == "ExternalOutput":
            shape = tuple(alloc.tensor_shape)
            dtype = mybir.dt.np(alloc.dtype)
            out_names.append(name)
            out_avals.append(jax.core.ShapedArray(shape, dtype))
            out_shapes.append((shape, dtype))

    n_params = len(in_names)
    n_outs = len(out_avals)
    all_names = list(in_names) + list(out_names)
    if partition_name is not None:
        all_names.append(partition_name)
    donate = tuple(range(n_params, n_params + n_outs))

    def _body(*args):
        operands = list(args)
        if partition_name is not None:
            operands.append(partition_id_tensor())
        outs = _bass_exec_p.bind(
            *operands,
            out_avals=tuple(out_avals),
            in_names=tuple(all_names),
            out_names=tuple(out_names),
            lowering_input_output_aliases=(),
            sim_require_finite=True,
            sim_require_nnan=True,
            nc=nc,
        )
        return tuple(outs)

    devices = jax.devices()[:NC]
    mesh = Mesh(np.asarray(devices), ("core",))
    in_specs = (PartitionSpec("core"),) * (n_params + n_outs)
    out_specs = (PartitionSpec("core"),) * len(out_names)
    sharded = jax.jit(
        shard_map(_body, mesh=mesh, in_specs=in_specs,
                  out_specs=out_specs, check_rep=False),
        donate_argnums=donate,
        keep_unused=True,
    )
    sharding = NamedSharding(mesh, PartitionSpec("core"))

    import jax.numpy as jnp

    zero_fns = [
        jax.jit(
            (lambda shape=shape, dtype=dtype:
             jnp.zeros((NC * shape[0], *shape[1:]), dtype)),
            out_shardings=sharding,
        )
        for shape, dtype in out_shapes
    ]

    class Runner:
        in_names = None
        out_names = None

        def stage(self, in_maps):
            import jax as _jax
            concat = [
                np.concatenate([np.asarray(in_maps[c][name])
                                for c in range(NC)], axis=0)
                for name in in_names
            ]
            staged = [_jax.device_put(a, sharding) for a in concat]
            _jax.block_until_ready(staged)
            return staged

        def make_zeros(self):
            return [zf() for zf in zero_fns]

        def run_async(self, staged, zs):
            return sharded(*staged, *zs)

        def run(self, staged):
            import jax as _jax
            outs = sharded(*staged, *self.make_zeros())
            _jax.block_until_ready(outs)
            return outs

    r = Runner()
    r.in_names = in_names
    r.out_names = out_names
    r.sharding = sharding
    return r


def _assemble(out_arr):
    """[NC*S, NPC] device/np array -> full [1, S, H] f32 output."""
    a = np.asarray(out_arr).reshape(NC, S, NPC)
    return a.transpose(1, 0, 2).reshape(1, S, H).astype(np.float32)


def kernel(positions, hidden_states, Wq, Wk, Wv, Wo):
    if "nc" not in _CACHE:
        _CACHE["nc"] = _build()
        _CACHE["runner"] = _make_runner(_CACHE["nc"])
    runner = _CACHE["runner"]
    in_maps = _host_prep(positions, hidden_states, Wq, Wk, Wv, Wo)
    staged = runner.stage(in_maps)
    outs = runner.run(staged)
    return _assemble(outs[0])
